# revision 19
# baseline (speedup 1.0000x reference)
"""Trainium2 Bass kernel for nn_DenseEntangler (B=256, D=32, L=3, 6 nodes).

Math: out = relu(bias + chain of 6 tensordot contractions). Each per-sample
contraction is a (1024 x 1024) matmul applied to the reshaped state, so the
whole problem is 6 matmuls of [1024,1024]^T @ [1024, Bc*32] per core
(Bc = 32 samples/core on 8 cores, batch-sharded).

Layout scheme (verified against the reference in numpy):
  state XT[(u*32+v) partition, (b*32+f) free], K = 1024 -> 8 tiles of 128.
  steps 0..4:  OUT[(n*32+m), (b,f)] = W_i^T @ XT  with
               W_i[(u*32+v), (n*32+m)] = nodes[i][u,v,m,n]  (host pre-permute)
               transition to the next step's XT = independent aligned 32x32
               block transposes (swap partition-low m with free-low f) ->
               native DVE stream_transpose, runs off the PE critical path.
  step 5:      operands swapped (state stationary, W5 moving) so PSUM comes
               out as [(b*32+f) partition, (m*32+n) free], which is
               DRAM-contiguous per partition for the final store.

Perf notes (v2): matmuls run in bf16 (1 cycle/row on the PE, identical to
float32r at N>=256, but FWL hides the weight loads and DMA bytes halve);
PSUM accumulation stays fp32. x is pre-permuted ON THE HOST into the exact
SBUF tile layout [k, p, b*f] so the head DMA is fully contiguous -- the
fp32r baseline lost ~50us to ~40GB/s strided gathers at the head. All six
weight sets are resident in SBUF (bf16 halves their footprint), loaded
up-front across the two HWDGE rings + gpsimd SWDGE.
"""

import os
import sys

import numpy as np

for _p in ("/opt/trn_rl_repo", "/root/.axon_site/_ro/trn_rl_repo"):
    if _p not in sys.path and os.path.isdir(_p):
        sys.path.append(_p)

B = 256
NCORES = 8
BC = B // NCORES  # 32 samples per core
NSTEP = 6
NK = 8  # K tiles of 128 (K = 1024)
NM = 8  # output partition tiles of 128 (steps 0..4)
NHALF = 2  # halves of 16 samples -> moving free dim 512
HB = BC // NHALF  # 16

_NC_CACHE = {}


def _np_dtype(mm_dtype_name):
    if mm_dtype_name in ("float32", "float32r"):
        return np.float32
    from ml_dtypes import bfloat16

    assert mm_dtype_name == "bfloat16", mm_dtype_name
    return bfloat16


def _build_nc(mm_dtype_name):
    import concourse.tile as tile
    from concourse import bacc, mybir

    f32 = mybir.dt.float32
    mmdt = getattr(mybir.dt, mm_dtype_name)
    # DRAM declaration dtype: f32 for the 4-byte paths (f32/f32r share bits),
    # bf16 natively otherwise. `cast` bitcasts an AP only when needed.
    ddt = f32 if mm_dtype_name in ("float32", "float32r") else mmdt
    cast = (lambda ap: ap.bitcast(mmdt)) if mmdt != ddt else (lambda ap: ap)

    # Bacc (not plain Bass): its lowering runs move_matmul_waits_to_ldweights
    # + generate_event_semaphores, required to satisfy the HW 1-wait-per-
    # instruction constraint on fused LDWEIGHTS+MATMUL.
    nc = bacc.Bacc(None)
    # x arrives pre-permuted from the host: x3[k, p, b*32+f] = x[b, (k*128+p)*32+f]
    xh = nc.declare_dram_parameter("x", [NK, 128, BC * 32], ddt, isOutput=False)
    wh = nc.declare_dram_parameter("w", [NSTEP, 128, NK * 1024], ddt, isOutput=False)
    bh = nc.declare_dram_parameter("bias_in", [32768], f32, isOutput=False)
    yh = nc.declare_dram_parameter("y", [BC, 32768], f32, isOutput=True)

    # bias[(f*1024 + q)] -> [f, q]
    b2 = bh[:].rearrange("(f q) -> f q", q=1024)
    # y[b, f*1024 + q] -> [b, f, q]
    y3 = yh[:, :].rearrange("b (f q) -> b f q", q=1024)

    with tile.TileContext(nc) as tc:
        with (
            tc.tile_pool(name="wpool", bufs=8) as wpool,
            tc.tile_pool(name="xpool", bufs=32) as xpool,
            tc.tile_pool(name="bpool", bufs=1) as bpool,
            tc.tile_pool(name="tpool", bufs=4) as tpool,
            tc.tile_pool(name="stpool", bufs=4) as stpool,
            tc.tile_pool(name="opool", bufs=4) as opool,
            tc.tile_pool(name="pspool", bufs=8, space="PSUM") as pspool,
        ):
            wsb = {s: [None] * NK for s in range(NSTEP)}

            # ---- PE pre-warm: the NEFF spends ~8.5us on startup (engine
            # barrier + iram fetch + DGE ring start) before the first data
            # DMA byte lands. Run junk matmuls on zeroed scratch tiles in
            # that window so the HAM throttle ramps to full clock (needs
            # ~3us of continuous PE busy) BEFORE the real operands arrive —
            # otherwise the first ~5us of real matmuls run at half clock.
            # HAM needs ~5.5us of GAP-free PE busy to reach full clock and a
            # >1us idle gap resets the ramp, so the junk stream must hand off
            # to the first real matmul with no gap: ladder of big-then-small
            # ops sized to end slightly AFTER the first (x, w0) pair lands
            # (~10.5-13us across runs) — overshoot costs its own length,
            # a ramp reset costs ~2.5us of half-clock real matmuls.
            jw = xpool.tile([128, 128], mmdt, tag="jw", bufs=1)
            jx = xpool.tile([128, 512], mmdt, tag="jx", bufs=1)
            nc.vector.memset(jw[:], 0.0)
            nc.vector.memset(jx[:], 0.0)
            jp = pspool.tile([128, 512], f32, tag="ps", name="junk_ps")
            # Producer-free 1x1 ops first (read the preamble's const APs, so
            # no waits at all): PE busy from the instant its stream opens,
            # ~1us before the memset-gated ladder below can start.
            cb1 = nc.const_aps.aps[(mybir.dt.bfloat16, 1.0)]
            for _ in range(30):
                nc.tensor.matmul(jp[0:1, 0:1], cb1, cb1, start=True, stop=True)
            for _ in range(5):
                nc.tensor.matmul(jp[:], jw[:], jx[:], start=True, stop=True)
            for _ in range(12):
                nc.tensor.matmul(jp[:, 0:128], jw[:], jx[:, 0:128], start=True, stop=True)

            # ---- head: interleave x[k] and w0[k] on the two HWDGE rings so
            # the PE's (x[k], w0[k]) pairs arrive in consumption order. All
            # transfers are contiguous 256KB; one dma_start per tile — each
            # trigger occupies its issuing engine ~0.65us, so fewer, larger
            # transfers beat finer pipelining here.
            x0 = [None] * NK
            for k in range(NK):
                qa, qb = (nc.sync, nc.scalar) if k % 2 == 0 else (nc.scalar, nc.sync)
                tx = xpool.tile([128, BC * 32], mmdt, tag="x0", name=f"x0_{k}", bufs=8)
                qa.dma_start(out=tx[:], in_=cast(xh[k, :, :]))
                x0[k] = tx
                t = wpool.tile([128, 1024], mmdt, tag="w", name=f"w0_{k}")
                qb.dma_start(out=t[:], in_=cast(wh[0, :, k * 1024 : (k + 1) * 1024]))
                wsb[0][k] = t

            # ---- all later weight sets, issued up-front but strictly BEHIND
            # the head tiles on the two FIFO HWDGE rings, ordered by the time
            # each step needs them. The head (x+w0, 4MB) gets the full
            # ~358GB/s HBM budget and drains in ~11us < step-0 compute; each
            # W_i (2MB per ring slot) lands long before its step starts.
            # gpsimd stays idle so SWDGE doesn't steal head bandwidth.
            # (tag note: a tag sizes to bufs*max(size), so the [128,8192]
            # chunks get their own tag "w8" with one buf per step 1..5.)
            def load_weights(step, eng):
                t = wpool.tile(
                    [128, NK * 1024], mmdt, tag="w8", bufs=5, name=f"w{step}"
                )
                eng.dma_start(out=t[:], in_=cast(wh[step, :, :]))
                for k in range(NK):
                    wsb[step][k] = t[:, k * 1024 : (k + 1) * 1024]

            load_weights(1, nc.sync)
            load_weights(2, nc.scalar)
            load_weights(3, nc.sync)
            load_weights(4, nc.scalar)
            load_weights(5, nc.sync)

            # bias tile: [128, 1024], row p holds bias[(p%32)*1024 : ...];
            # queued behind the weight stream so the sync ring is free for
            # the step-5 stores by the time they start.
            bias_sb = bpool.tile([128, 1024], f32, tag="bias")
            for r in range(4):
                nc.sync.dma_start(out=bias_sb[32 * r : 32 * (r + 1), :], in_=b2[:, :])

            def finish_tile(ps, h, mt, xt_next):
                """PSUM -> (transpose, round-to-mmdt) -> next-step state tile."""
                if mmdt is f32:
                    t = xpool.tile([128, 512], f32, tag="xt")
                    nc.vector.transpose(t[:], ps[:])
                else:
                    st = stpool.tile([128, 512], f32, tag="st")
                    nc.vector.transpose(st[:], ps[:])
                    t = xpool.tile([128, 512], mmdt, tag="xt")
                    nc.scalar.copy(t[:], st[:])
                xt_next[h][mt] = t

            # ---- step 0: k-outer so PE consumes k-tiles in DMA arrival order
            xt_next = [[None] * NK for _ in range(NHALF)]
            for h in range(NHALF):
                pss = [
                    pspool.tile([128, 512], f32, tag="ps", name=f"ps0_{h}_{i}")
                    for i in range(NM)
                ]
                for k in range(NK):
                    for mt in range(NM):
                        nc.tensor.matmul(
                            pss[mt][:],
                            wsb[0][k][:, mt * 128 : (mt + 1) * 128],
                            x0[k][:, h * 512 : (h + 1) * 512],
                            start=(k == 0),
                            stop=(k == NK - 1),
                        )
                for mt in range(NM):
                    finish_tile(pss[mt], h, mt, xt_next)
            xt = xt_next

            # ---- steps 1..4: mt-outer (staggers transposes across the step)
            for step in range(1, 5):
                xt_next = [[None] * NK for _ in range(NHALF)]
                for h in range(NHALF):
                    for mt in range(NM):
                        ps = pspool.tile([128, 512], f32, tag="ps")
                        for k in range(NK):
                            nc.tensor.matmul(
                                ps[:],
                                wsb[step][k][:, mt * 128 : (mt + 1) * 128],
                                xt[h][k][:],
                                start=(k == 0),
                                stop=(k == NK - 1),
                            )
                        finish_tile(ps, h, mt, xt_next)
                xt = xt_next

            # ---- step 5: state stationary, W moving; fused bias+relu+store ----
            from concourse.mybir import ActivationFunctionType

            for h in range(NHALF):
                for mc in range(4):  # output partition chunks of 128 (= 4 b values)
                    for nh in range(2):  # N halves of 512
                        ps = pspool.tile([128, 512], f32, tag="ps")
                        for k in range(NK):
                            nc.tensor.matmul(
                                ps[:],
                                xt[h][k][:, mc * 128 : (mc + 1) * 128],
                                wsb[5][k][:, nh * 512 : (nh + 1) * 512],
                                start=(k == 0),
                                stop=(k == NK - 1),
                            )
                        # bias add on DVE; relu alternates DVE/ACT so neither
                        # engine backs up behind the last matmul (gpsimd's
                        # vector ops are Q7-software, ~7us each — unusable).
                        # The last group is chunked in two with stores on both
                        # rings so the end-of-kernel serial chain is short.
                        last = h == NHALF - 1 and mc == 3 and nh == 1
                        nchunk = 2 if last else 1
                        cw = 512 // nchunk
                        tmp = tpool.tile([128, 512], f32, tag="tmp")
                        o = opool.tile([128, 512], f32, tag="o")
                        b0 = h * HB + mc * 4
                        for ci in range(nchunk):
                            sl = slice(ci * cw, (ci + 1) * cw)
                            nc.vector.tensor_add(
                                tmp[:, sl],
                                ps[:, sl],
                                bias_sb[:, nh * 512 + ci * cw : nh * 512 + (ci + 1) * cw],
                            )
                            if (mc + nh) % 2 == 1 and not last:
                                nc.scalar.activation(
                                    o[:, sl], tmp[:, sl], ActivationFunctionType.Relu
                                )
                            else:
                                nc.vector.tensor_scalar_max(o[:, sl], tmp[:, sl], 0.0)
                            eng = nc.sync if (mc + nh + ci) % 2 == 0 else nc.scalar
                            eng.dma_start(
                                out=y3[
                                    b0 : b0 + 4, :, nh * 512 + ci * cw : nh * 512 + (ci + 1) * cw
                                ],
                                in_=o[:, sl],
                            )
    # Run the Bacc lowering passes (register allocation, wait splitting, ...)
    # — the PJRT execute path serializes nc.m as-is.
    nc.finalize()
    return nc


def _get_nc(mm_dtype_name):
    if mm_dtype_name not in _NC_CACHE:
        _NC_CACHE[mm_dtype_name] = _build_nc(mm_dtype_name)
    return _NC_CACHE[mm_dtype_name]


def _prep_weights(nodes, npdt):
    # W[i] layout [p=(u*32+v)%... rows 128 per k-tile packed as [128, 8*1024]]:
    # free index = k*1024 + col.  steps 0..4: col = n*32+m ; step 5: col = m*32+n.
    nodes = np.ascontiguousarray(nodes, dtype=np.float32)
    W = np.empty((NSTEP, 128, 8192), np.float32)
    for i in range(NSTEP):
        if i < 5:
            wm = nodes[i].reshape(1024, 32, 32).transpose(0, 2, 1).reshape(1024, 1024)
        else:
            wm = nodes[i].reshape(1024, 1024)
        # [k*128+p, col] -> [p, k*1024+col]
        W[i] = wm.reshape(NK, 128, 1024).transpose(1, 0, 2).reshape(128, 8192)
    return np.ascontiguousarray(W.astype(npdt))


def _prep_x(xc, npdt):
    # [BC, 32768] -> [k, p, b*32+f] with value x[b, (k*128+p)*32+f]
    xp = xc.reshape(BC, NK, 128, 32).transpose(1, 2, 0, 3).reshape(NK, 128, BC * 32)
    return np.ascontiguousarray(xp.astype(npdt))


def run(inputs, nodes, bias, mm_dtype="bfloat16", trace=False):
    from concourse.bass_utils import run_bass_kernel_spmd

    nc = _get_nc(mm_dtype)
    npdt = _np_dtype(mm_dtype)
    x = np.ascontiguousarray(inputs, dtype=np.float32)
    bias = np.ascontiguousarray(bias, dtype=np.float32)
    W = _prep_weights(nodes, npdt)
    in_maps = [
        {
            "x": _prep_x(x[c * BC : (c + 1) * BC], npdt),
            "w": W,
            "bias_in": bias,
        }
        for c in range(NCORES)
    ]
    res = run_bass_kernel_spmd(nc, in_maps, list(range(NCORES)), trace=trace)
    out = np.concatenate([res.results[c]["y"] for c in range(NCORES)], axis=0)
    return out, res


def kernel(inputs, nodes, bias):
    out, _ = run(inputs, nodes, bias)
    return out
